# revision 26
# baseline (speedup 1.0000x reference)
"""Bass/Trainium2 kernel for batched dot-product attention.

Problem: q,k,v [B=4, S=4096, D=1024]; projections to dk=dv=128; softmax
attention per batch element.  Sharded over 8 NeuronCores as (batch,
key-half): core c handles batch c//2 with keys (c%2)*2048 ... +2048 and
ALL 4096 queries, producing unnormalized partial AV outputs plus partial
exp-sums; the host merges the two key-halves (flash-attention style
without max subtraction -- scores are small, exp is safe in bf16).

Versus the query-split layout this deduplicates the kp/vp projections
(each key is projected once instead of twice); qp is duplicated instead,
which is half the size, and q/k projections run in fp8 (DoubleRow, 2x)
making the duplication cheap.

On-chip layouts keep the contraction dim on SBUF partitions:
  qT/kT/vT  [d_model, seq]   (host pre-transposed; q/k fp8e4, v bf16)
  qpT/kpT   [dk, seq]        (bf16)
  vp        [seq, dv]        (natural layout via PE transposes, bf16)
  S^T tiles [keys, q]        (scores transposed, PSUM f32)
  out^T     [dv, q]          (partial AV, bf16; host merges + undoes)

fp8 notes: wq/wk are boosted by a power of two before e4m3 quantization
(else they land in the subnormal range) and de-boosted in the fused
scale+bias step off PSUM.  Scale 1/sqrt(dk) is folded into wq/bq.

Softmax denominators: running chain of bf16 adds over the 16 exp tiles
of each query block (DVE 2x mode), shipped as a [128, 4096] partial-sum
plane; host reduces the final 128 partitions.
"""

import math

import numpy as np
import ml_dtypes

import concourse.bass as bass
import concourse.tile as tile
from concourse import bacc, mybir
from concourse.bass_utils import run_bass_kernel_spmd

B, S, DM, DK, DV = 4, 4096, 1024, 128, 128
N_CORES = 8
KH = S // 2          # keys per core (2048)
NKC = KH // 128      # key chunks of 128 per core (16)
NKB = KH // 512      # key blocks of 512 per core (4)
NQB = S // 1024      # query blocks of 1024 (4)
NQPB = S // 512      # qp projection blocks of 512 (8)
NMC = DM // 128      # d_model chunks (8)

FP8_QK = True        # q/k projections in fp8e4 DoubleRow (2x PE, half DMA)
WQ_BOOST = 128.0     # power-of-2 pre-quantization boost for wq*scale
WK_BOOST = 32.0      # same for wk

BF16 = mybir.dt.bfloat16
F32 = mybir.dt.float32
E4 = mybir.dt.float8e4
NP_BF16 = ml_dtypes.bfloat16
NP_E4 = ml_dtypes.float8_e4m3fn

QK_DT = E4 if FP8_QK else BF16
NP_QK = NP_E4 if FP8_QK else NP_BF16

Exp = mybir.ActivationFunctionType.Exp
DoubleRow = mybir.MatmulPerfMode.DoubleRow


def _emit(tc: tile.TileContext, aps: dict):
    nc = tc.nc
    qT, kT, vT = aps["qT"], aps["kT"], aps["vT"]
    outT, accT = aps["outT"], aps["accT"]

    with tc.tile_pool(name="persist", bufs=1) as persist:
        # --- constants: critical ones (bias, wk, wq) first on the scalar
        # queue; wv on sync (not needed until vp0, ~15us in)
        # bias_pack is padded to 512B partition lines: a [128, 4] transfer
        # degenerates to 4-byte DMA packets that trickle in for ~15us and
        # gate the entire first-exp chain
        bias_sb = persist.tile([128, 128], F32, tag="bias")
        nc.sync.dma_start(bias_sb[:], aps["bias_pack"][:])
        bq_ap, bk_ap, bv_ap = bias_sb[:, 0:1], bias_sb[:, 1:2], bias_sb[:, 2:3]
        ident_sb = persist.tile([128, 128], BF16, tag="ident")
        nc.sync.dma_start(ident_sb[:], aps["ident"][:])
        w_sb = {}
        for name, dt in (("wk", QK_DT), ("wq", QK_DT), ("wv", BF16)):
            t = persist.tile([128, NMC, 128], dt, tag=f"w_{name}", name=f"w_{name}")
            nc.scalar.dma_start(t[:], aps[name][:])
            w_sb[name] = t

        # --- persistent activations ---
        kpT = persist.tile([128, NKB, 512], BF16, tag="kpT", name="kpT")
        qpT = persist.tile([128, NQPB, 512], BF16, tag="qpT", name="qpT")
        vp = persist.tile([128, NKC, 128], BF16, tag="vp", name="vp")

        with (
            tc.tile_pool(name="pp", bufs=2, space="PSUM") as pp,
            tc.tile_pool(name="sp", bufs=2, space="PSUM") as sp,
            tc.tile_pool(name="ovp", bufs=1, space="PSUM") as ovp,
            tc.tile_pool(name="xs", bufs=2) as xs,
            tc.tile_pool(name="ep", bufs=6) as ep,
            tc.tile_pool(name="trp", bufs=2) as trp,
            tc.tile_pool(name="outp", bufs=2) as outp,
        ):
            # ---- input fetch (gpsimd queue: near-free issue) ----
            kxs, vxs, qxs = {}, {}, {}

            def fetch(store, src, blk, dt, tag, bufs, parts, eng):
                t = xs.tile([128, NMC, 512], dt, tag=tag, name=f"{tag}{blk}",
                            bufs=bufs)
                step = NMC // parts
                for i in range(0, NMC, step):
                    eng.dma_start(t[:, i:i + step, :],
                                  src[blk][:, i:i + step, :])
                store[blk] = t

            def fetch_kx(blk, parts=1):
                fetch(kxs, kT, blk, QK_DT, "kx", 2, parts, nc.gpsimd)

            def fetch_qx(blk, parts=1):
                fetch(qxs, qT, blk, QK_DT, "qx", 3, parts, nc.gpsimd)

            def fetch_vx(blk, parts=2):
                fetch(vxs, vT, blk, BF16, "vx", 2, parts, nc.gpsimd)

            # ---- projections ----
            def proj_fp8(w, x, ps, c0, c1):
                for c in range(c0, c1, 2):
                    nc.tensor.matmul(
                        ps[:], lhsT=w[:, c:c + 2, :], rhs=x[:, c:c + 2, :],
                        start=(c == 0), stop=(c == NMC - 2),
                        perf_mode=DoubleRow,
                    )

            def proj_bf16(w, x, ps, c0, c1):
                for c in range(c0, c1):
                    nc.tensor.matmul(
                        ps[:], lhsT=w[:, c, :], rhs=x[:, c, :],
                        start=(c == 0), stop=(c == NMC - 1),
                    )

            proj_qk = proj_fp8 if FP8_QK else proj_bf16

            def proj_qp(qb, c0=0, c1=NMC, _ps={}):
                if c0 == 0:
                    _ps[qb] = pp.tile([128, 512], F32, tag="pp", name=f"psq{qb}")
                proj_qk(w_sb["wq"], qxs[qb][:], _ps[qb][:], c0, c1)
                if c1 == NMC:
                    qxs.pop(qb)
                    if FP8_QK:
                        nc.vector.tensor_scalar(
                            qpT[:, qb, :], _ps.pop(qb)[:], 1.0 / WQ_BOOST,
                            bq_ap, mybir.AluOpType.mult, mybir.AluOpType.add)
                    else:
                        nc.vector.tensor_scalar_add(
                            qpT[:, qb, :], _ps.pop(qb)[:], bq_ap)

            def proj_kp(kb, c0=0, c1=NMC, _ps={}):
                if c0 == 0:
                    _ps[kb] = pp.tile([128, 512], F32, tag="pp", name=f"psk{kb}")
                proj_qk(w_sb["wk"], kxs[kb][:], _ps[kb][:], c0, c1)
                if c1 == NMC:
                    kxs.pop(kb)
                    if FP8_QK:
                        nc.vector.tensor_scalar(
                            kpT[:, kb, :], _ps.pop(kb)[:], 1.0 / WK_BOOST,
                            bk_ap, mybir.AluOpType.mult, mybir.AluOpType.add)
                    else:
                        nc.vector.tensor_scalar_add(
                            kpT[:, kb, :], _ps.pop(kb)[:], bk_ap)

            def proj_vp(kb, c0=0, c1=NMC, _ps={}):
                if c0 == 0:
                    _ps[kb] = pp.tile([128, 512], F32, tag="pp", name=f"psv{kb}")
                proj_bf16(w_sb["wv"], vxs[kb][:], _ps[kb][:], c0, c1)
                if c1 < NMC:
                    return
                vxs.pop(kb)
                vpt = xs.tile([128, 512], BF16, tag="vpt", name=f"vpt{kb}")
                nc.vector.tensor_scalar_add(vpt[:], _ps.pop(kb)[:], bv_ap)
                for j in range(2):
                    tp = pp.tile([128, 256], BF16, tag="pp", name=f"tp{kb}_{j}")
                    for i in range(2):
                        nc.tensor.transpose(
                            tp[:, i * 128:(i + 1) * 128],
                            vpt[:, (2 * j + i) * 128:(2 * j + i + 1) * 128],
                            ident_sb[:],
                        )
                    nc.vector.tensor_copy(vp[:, 4 * kb + 2 * j:4 * kb + 2 * j + 2, :],
                                          tp[:])

            # ---- attention machinery (per query block of 1024) ----
            class QB:
                def __init__(self, qb):
                    self.qb = qb
                    self.ov = ovp.tile([128, 1024], F32, tag="ov", name=f"ov{qb}")
                    self.pend = []       # (kc, s_psum) awaiting exp
                    self.av_pend = []    # (kc, e) awaiting AV matmul
                    self.acc = None      # running bf16 sum of exp tiles

                def scores(self, kc, halves=(0, 1)):
                    if not self.pend or self.pend[-1][0] != kc:
                        s = sp.tile([128, 1024], F32, tag="sp",
                                    name=f"s{self.qb}_{kc}")
                        self.pend.append((kc, s))
                    s = self.pend[-1][1]
                    ks = kpT[:, kc // 4, (kc % 4) * 128:(kc % 4 + 1) * 128]
                    for h in halves:
                        nc.tensor.matmul(
                            s[:, h * 512:(h + 1) * 512], lhsT=ks,
                            rhs=qpT[:, 2 * self.qb + h, :],
                            start=True, stop=True,
                        )

                def exp(self, halves=None):
                    kc, s = self.pend.pop(0)
                    e = ep.tile([128, 1024], BF16, tag="e",
                                name=f"e{self.qb}_{kc}")
                    if halves is None:
                        nc.scalar.activation(e[:], s[:], Exp)
                    else:
                        for h in halves:
                            nc.scalar.activation(
                                e[:, h * 512:(h + 1) * 512],
                                s[:, h * 512:(h + 1) * 512], Exp)
                    self.av_pend.append((kc, e))
                    if self.acc is None:
                        self.acc = e
                    else:
                        nt = trp.tile([128, 1024], BF16, tag="chain",
                                      name=f"c{self.qb}_{kc}", bufs=2)
                        nc.vector.tensor_add(nt[:], self.acc[:], e[:])
                        self.acc = nt

                def av(self):
                    kc, e = self.av_pend.pop(0)
                    for h in range(2):
                        nc.tensor.matmul(
                            self.ov[:, h * 512:(h + 1) * 512],
                            lhsT=vp[:, kc, :],
                            rhs=e[:, h * 512:(h + 1) * 512],
                            start=(kc == 0), stop=(kc == NKC - 1),
                        )

                def drain(self):
                    nc.sync.dma_start(
                        accT[:, self.qb * 1024:(self.qb + 1) * 1024],
                        self.acc[:])
                    outsb = outp.tile([128, 1024], BF16, tag="out",
                                      name=f"out{self.qb}")
                    qlo = self.qb * 1024
                    nc.vector.tensor_copy(outsb[:], self.ov[:])
                    nc.sync.dma_start(outT[:, qlo:qlo + 1024], outsb[:])

                def final_flush(self):
                    # last block: interleave the last AV's halves with the
                    # psum->sbuf copies (Scalar, idle by now) and the out
                    # DMAs so the post-AV chain is as short as possible
                    while len(self.av_pend) > 1:
                        self.av()
                    nc.sync.dma_start(
                        accT[:, self.qb * 1024:(self.qb + 1) * 1024],
                        self.acc[:])
                    kc, e = self.av_pend.pop(0)
                    outsb = outp.tile([128, 1024], BF16, tag="out",
                                      name=f"out{self.qb}")
                    qlo = self.qb * 1024
                    for h in range(2):
                        nc.tensor.matmul(
                            self.ov[:, h * 512:(h + 1) * 512],
                            lhsT=vp[:, kc, :],
                            rhs=e[:, h * 512:(h + 1) * 512],
                            start=(kc == 0), stop=(kc == NKC - 1),
                        )
                        nc.scalar.activation(
                            outsb[:, h * 512:(h + 1) * 512],
                            self.ov[:, h * 512:(h + 1) * 512],
                            mybir.ActivationFunctionType.Copy)
                        nc.sync.dma_start(
                            outT[:, qlo + h * 512:qlo + (h + 1) * 512],
                            outsb[:, h * 512:(h + 1) * 512])

            # ---- schedule ----
            # Input stream: kx0/qx0/qx1 lead finely chunked so the first
            # scores+exp start as soon as data lands; vx0 follows.
            fetch_kx(0)
            fetch_qx(0)
            fetch_qx(1)
            fetch_vx(0)
            fetch_kx(1)
            fetch_vx(1)
            fetch_kx(2)
            fetch_vx(2)
            fetch_kx(3)
            fetch_vx(3)
            for qb in range(2, NQPB):
                fetch_qx(qb)

            # PE warmup: the head is DMA-bound for ~4us after the ident
            # tile lands; dummy matmuls keep the PE executing so its DVFS
            # clock ramps before the real projections start.
            warm = pp.tile([128, 128], F32, tag="pp", name="warm")
            for _ in range(3):
                nc.tensor.matmul(warm[:], lhsT=ident_sb[:], rhs=ident_sb[:],
                                 start=True, stop=True)

            # Critical prefix: first exp fires after kp0 + qp0 (h0 half)
            # then qp1 completes the tile.
            st0 = QB(0)
            proj_kp(0)
            proj_qp(0)
            st0.scores(0, halves=(0,))
            proj_qp(1)
            st0.scores(0, halves=(1,))
            st0.exp(halves=(0, 1))
            st0.scores(1)
            st0.exp()
            proj_vp(0)

            # Detours: PE work emitted between attention chunks of qb0/qb1,
            # each at most ~1.8us so the 2-deep scores lookahead keeps the
            # exp chain fed.
            half = NMC // 2
            detours = {
                (0, 2): [lambda: proj_kp(1, 0, half)],
                (0, 3): [lambda: proj_kp(1, half, NMC)],
                (0, 4): [lambda: proj_vp(1, 0, half)],
                (0, 5): [lambda: proj_vp(1, half, NMC)],
                (0, 6): [lambda: proj_kp(2, 0, half)],
                (0, 7): [lambda: proj_kp(2, half, NMC)],
                (0, 8): [lambda: proj_vp(2, 0, half)],
                (0, 9): [lambda: proj_vp(2, half, NMC)],
                (0, 10): [lambda: proj_kp(3, 0, half)],
                (0, 11): [lambda: proj_kp(3, half, NMC)],
                (0, 12): [lambda: proj_vp(3, 0, half),
                          lambda: proj_qp(2, 0, half)],
                (0, 13): [lambda: proj_vp(3, half, NMC),
                          lambda: proj_qp(2, half, NMC)],
                (0, 14): [lambda: proj_qp(3, 0, half)],
                (0, 15): [lambda: proj_qp(3, half, NMC)],
                (1, 2): [lambda: proj_qp(4, 0, half)],
                (1, 3): [lambda: proj_qp(4, half, NMC)],
                (1, 6): [lambda: proj_qp(5, 0, half)],
                (1, 7): [lambda: proj_qp(5, half, NMC)],
                (2, 2): [lambda: proj_qp(6, 0, half)],
                (2, 3): [lambda: proj_qp(6, half, NMC)],
                (2, 6): [lambda: proj_qp(7, 0, half)],
                (2, 7): [lambda: proj_qp(7, half, NMC)],
            }

            # All blocks run their AV matmuls 2 chunks behind the exps;
            # the 2 leftover AVs of block N flush during block N+1's first
            # two chunks so scores production never pauses at boundaries.
            prev = None
            for qb in range(NQB):
                st = st0 if qb == 0 else QB(qb)
                for kc in range(2 if qb == 0 else 0, NKC):
                    st.scores(kc)
                    st.exp()
                    for d in detours.get((qb, kc), ()):
                        d()
                    if kc >= 2:
                        st.av()
                    if prev is not None and kc <= 1:
                        prev.av()
                        if kc == 1:
                            prev.drain()
                            prev = None
                if qb == NQB - 1:
                    st.final_flush()
                else:
                    prev = st


_CACHE = {}


def _build():
    if "nc" in _CACHE:
        return _CACHE["nc"]
    nc = bacc.Bacc("TRN2", debug=False, num_devices=N_CORES)
    aps = {
        "qT": nc.dram_tensor("qT", [NQPB, 128, NMC, 512], QK_DT,
                             kind="ExternalInput").ap(),
        "kT": nc.dram_tensor("kT", [NKB, 128, NMC, 512], QK_DT,
                             kind="ExternalInput").ap(),
        "vT": nc.dram_tensor("vT", [NKB, 128, NMC, 512], BF16,
                             kind="ExternalInput").ap(),
        "wq": nc.dram_tensor("wq", [128, NMC, DK], QK_DT,
                             kind="ExternalInput").ap(),
        "wk": nc.dram_tensor("wk", [128, NMC, DK], QK_DT,
                             kind="ExternalInput").ap(),
        "wv": nc.dram_tensor("wv", [128, NMC, DV], BF16,
                             kind="ExternalInput").ap(),
        "bias_pack": nc.dram_tensor(
            "bias_pack", [128, 128], F32, kind="ExternalInput").ap(),
        "ident": nc.dram_tensor("ident", [128, 128], BF16,
                                kind="ExternalInput").ap(),
        "outT": nc.dram_tensor("outT", [DV, S], BF16,
                               kind="ExternalOutput").ap(),
        "accT": nc.dram_tensor("accT", [128, S], BF16,
                               kind="ExternalOutput").ap(),
    }
    with tile.TileContext(nc) as tc:
        _emit(tc, aps)
    nc.compile()
    _CACHE["nc"] = nc
    return nc


def _pack_w(w, np_dt):
    # [DM, d] -> [128, NMC, d]  (chunk-major weight layout)
    return np.ascontiguousarray(
        np.asarray(w).reshape(NMC, 128, -1).transpose(1, 0, 2)).astype(np_dt)


def _pack_x(xT, nblk, np_dt):
    # [DM, n] -> [nblk, 128, NMC, 512]  (contiguous per-stripe layout)
    return np.ascontiguousarray(
        xT.reshape(NMC, 128, nblk, 512).transpose(2, 1, 0, 3)).astype(np_dt)


def make_in_maps(q, k, v, wq, bq, wk, bk, wv, bv):
    scale = 1.0 / math.sqrt(DK)
    if FP8_QK:
        wq_p = _pack_w(np.asarray(wq, np.float32) * (scale * WQ_BOOST), NP_QK)
        wk_p = _pack_w(np.asarray(wk, np.float32) * WK_BOOST, NP_QK)
    else:
        wq_p = _pack_w(np.asarray(wq, np.float32) * scale, NP_QK)
        wk_p = _pack_w(np.asarray(wk, np.float32), NP_QK)
    wv_p = _pack_w(np.asarray(wv, np.float32), NP_BF16)
    bias_pack = np.zeros((128, 128), np.float32)
    bias_pack[:, 0] = np.asarray(bq, np.float32) * scale
    bias_pack[:, 1] = np.asarray(bk, np.float32)
    bias_pack[:, 2] = np.asarray(bv, np.float32)
    ident = np.eye(128, dtype=NP_BF16)
    in_maps = []
    qT_cache = {}
    for core in range(N_CORES):
        b, h = core // 2, core % 2
        if b not in qT_cache:
            qT_cache[b] = _pack_x(np.asarray(q[b], np.float32).T, NQPB, NP_QK)
        kTb = _pack_x(
            np.asarray(k[b], np.float32).T[:, h * KH:(h + 1) * KH], NKB, NP_QK)
        vTb = _pack_x(
            np.asarray(v[b], np.float32).T[:, h * KH:(h + 1) * KH], NKB, NP_BF16)
        in_maps.append({
            "qT": qT_cache[b], "kT": kTb, "vT": vTb,
            "wq": wq_p, "wk": wk_p, "wv": wv_p,
            "bias_pack": bias_pack, "ident": ident,
        })
    return in_maps


def kernel(q, k, v, wq, bq, wk, bk, wv, bv, _trace=False, _tmpdir=None):
    nc = _build()
    in_maps = make_in_maps(q, k, v, wq, bq, wk, bk, wv, bv)
    res = run_bass_kernel_spmd(
        nc, in_maps, list(range(N_CORES)), trace=_trace, tmpdir=_tmpdir
    )
    out = np.empty((B, S, DV), np.float32)
    for b in range(B):
        r0, r1 = res.results[2 * b], res.results[2 * b + 1]
        o = r0["outT"].astype(np.float32) + r1["outT"].astype(np.float32)
        sums = (r0["accT"].astype(np.float32).sum(axis=0)
                + r1["accT"].astype(np.float32).sum(axis=0))
        out[b] = (o / sums[None, :]).T
    if _trace:
        kernel.last_results = res
    return out
